# revision 8
# baseline (speedup 1.0000x reference)
"""Trainium2 Bass kernel for nn_MultiHeadAttention (B=4, S=2048, D=512, H=8, DK=12, DV=32).

Sharding: 8 cores = (batch b in 0..3) x (head-group g in 0..1, 4 heads each).
Each core computes, for its batch and its 4 heads:
  q/k/v projections, scores, masked softmax, p_attn (returned), x = p @ v,
  and a PARTIAL output projection x @ Wx[group rows].
Host sums the two partial outputs per batch (the "all-reduce") and adds bx.

Per-core kernel design notes:
  * All matmul operands are bf16 (fp32 matmuls are split into LO/HI passes on
    TRN2 at 2x the cost). Scores/exp/softmax normalization stay fp32.
  * Scores use a K=13 contraction: rows 0-11 are head dims, row 12 carries
    ones (qT side) x mask-bias (-1e9 or 0, kT side) so masking is folded into
    the score matmul for free.
  * exp runs on the scalar engine over a whole [128, 2048] stripe with
    scale=1/sqrt(DK) and accum_out producing row sums; softmax skips the
    max-subtraction (scores are O(1); exp((s-1e9)*scale) underflows to 0.0
    exactly like the reference).
  * p is normalized on DVE, DMA'd out in fp32, converted to bf16, and
    transposed via the DMA xbar (2-byte dtype) straight into SBUF as the
    moving operand of the x = p @ v matmul. No PE transposes, no PSUM copies.
"""

import math
from contextlib import ExitStack

import ml_dtypes
import numpy as np

import concourse.bass as bass
import concourse.tile as tile
from concourse import bacc, mybir
from concourse.bass import ts, ds
from concourse.bass_utils import run_bass_kernel_spmd
from concourse.masks import make_identity

AF = mybir.ActivationFunctionType
ALU = mybir.AluOpType
F32 = mybir.dt.float32
BF16 = mybir.dt.bfloat16
NPBF = ml_dtypes.bfloat16

B, S, D = 4, 2048, 512
H, DK, DV = 8, 12, 32
HPC = 4  # heads per core
NCORES = 8
P = 128
ISCALE = 1.0 / math.sqrt(DK)
NEG = -1.0e9


def _emit(ctx: ExitStack, tc: "tile.TileContext", io: dict, s: int) -> None:
    nc = tc.nc
    nt = s // P          # q/k row tiles
    nch = s // 512       # 512-wide column chunks of the score row
    dch = D // P         # contraction chunks over D

    consts = ctx.enter_context(tc.tile_pool(name="consts", bufs=1))
    persist = ctx.enter_context(tc.tile_pool(name="persist", bufs=1))

    wq_sb = consts.tile([P, dch, HPC * DK], BF16)
    nc.sync.dma_start(wq_sb, io["wq"].rearrange("(c p) m -> p c m", p=P))
    wk_sb = consts.tile([P, dch, HPC * DK], BF16)
    nc.sync.dma_start(wk_sb, io["wk"].rearrange("(c p) m -> p c m", p=P))
    wv_sb = consts.tile([P, dch, HPC * DV], BF16)
    nc.sync.dma_start(wv_sb, io["wv"].rearrange("(c p) m -> p c m", p=P))
    wx_sb = consts.tile([P, D], BF16)
    nc.sync.dma_start(wx_sb, io["wx"])
    bq_sb = consts.tile([DK, HPC], F32)
    nc.sync.dma_start(bq_sb, io["bq"])
    bk_sb = consts.tile([DK, HPC], F32)
    nc.sync.dma_start(bk_sb, io["bk"])
    bv_sb = consts.tile([1, HPC * DV], BF16)
    nc.sync.dma_start(bv_sb, io["bv"])
    ones_sb = consts.tile([1, P], BF16)
    nc.vector.memset(ones_sb, 1.0)
    ident = consts.tile([P, P], F32)
    make_identity(nc, ident)

    # qTe/kTe: rows 0..11 = projected head dims (transposed), row 12 = ones /
    # mask-bias so the score matmul applies the mask.
    qTe = persist.tile([DK + 1, HPC, s], BF16)
    kTe = persist.tile([DK + 1, HPC, s], BF16)
    v4 = persist.tile([P, nt, HPC * DV], BF16)   # [k_local, k_tile, head*dv]
    xT4 = persist.tile([P, s], BF16)             # [head*dv, q]

    for h in range(HPC):
        nc.sync.dma_start(qTe[DK : DK + 1, h, :], io["mb"][1:2, :])
        nc.sync.dma_start(kTe[DK : DK + 1, h, :], io["mb"][0:1, :])

    # ---- Stage A: transpose inputs, project q/k/v ----
    with tc.tile_pool(name="loads", bufs=3) as loads, \
         tc.tile_pool(name="rawT", bufs=2) as rawT, \
         tc.tile_pool(name="psumA", bufs=2, space="PSUM") as psumA, \
         tc.tile_pool(name="psumP", bufs=2, space="PSUM") as psumP:

        def transpose_in(src):
            """Load [s, D] DRAM tensor, return SBUF [P, dch, s] bf16 transpose."""
            dstT = rawT.tile([P, dch, s], BF16, tag="rawT")
            for t in range(nt):
                xt = loads.tile([P, D], F32, tag="xt")
                nc.gpsimd.dma_start(xt, src[ts(t, P), :])
                tp = psumA.tile([P, dch, P], F32, tag="tp")
                for c in range(dch):
                    nc.tensor.transpose(tp[:, c, :], xt[:, ts(c, P)], ident)
                nc.vector.tensor_copy(dstT[:, :, ts(t, P)], tp)
            return dstT

        qraw = transpose_in(io["xq"])
        for h in range(HPC):
            for n in range(s // 512):
                pq = psumP.tile([DK, 512], F32, tag="pqk")
                for c in range(dch):
                    nc.tensor.matmul(
                        pq, wq_sb[:, c, ds(h * DK, DK)], qraw[:, c, ds(n * 512, 512)],
                        start=(c == 0), stop=(c == dch - 1))
                nc.scalar.activation(
                    qTe[:DK, h, ds(n * 512, 512)], pq, AF.Identity,
                    bias=bq_sb[:, h : h + 1], scale=1.0)

        kraw = transpose_in(io["xk"])
        for h in range(HPC):
            for n in range(s // 512):
                pk = psumP.tile([DK, 512], F32, tag="pqk")
                for c in range(dch):
                    nc.tensor.matmul(
                        pk, wk_sb[:, c, ds(h * DK, DK)], kraw[:, c, ds(n * 512, 512)],
                        start=(c == 0), stop=(c == dch - 1))
                nc.scalar.activation(
                    kTe[:DK, h, ds(n * 512, 512)], pk, AF.Identity,
                    bias=bk_sb[:, h : h + 1], scale=1.0)

        vraw = transpose_in(io["xv"])
        for t in range(nt):
            pv = psumP.tile([P, HPC * DV], F32, tag="pv")
            for c in range(dch):
                nc.tensor.matmul(pv, vraw[:, c, ts(t, P)], wv_sb[:, c, :],
                                 start=(c == 0), stop=False)
            nc.tensor.matmul(pv, ones_sb, bv_sb, start=False, stop=True)
            nc.vector.tensor_copy(v4[:, t, :], pv)

    # ---- Stage B: attention per (head, q-stripe) ----
    with tc.tile_pool(name="pbuf", bufs=3) as pbuf, \
         tc.tile_pool(name="pobuf", bufs=4) as pobuf, \
         tc.tile_pool(name="pbbuf", bufs=3) as pbbuf, \
         tc.tile_pool(name="ptbuf", bufs=2) as ptbuf, \
         tc.tile_pool(name="stats", bufs=8) as stats, \
         tc.tile_pool(name="psumS", bufs=3, space="PSUM") as psumS, \
         tc.tile_pool(name="psumX", bufs=2, space="PSUM") as psumX:
        for h in range(HPC):
            pts4 = None
            for qt in range(nt):
                nhf = max(1, s // 1024)
                sums = stats.tile([P, nhf], F32, tag="sums")
                ptil = pbuf.tile([P, s], BF16, tag="ptil")
                for hf in range(nhf):
                    hw = min(1024, s)
                    sc = psumS.tile([P, 1024], F32, tag="sc")
                    for c in range(hw // 512):
                        nc.tensor.matmul(
                            sc[:, ds(c * 512, 512)],
                            qTe[:, h, ts(qt, P)],
                            kTe[:, h, ds(hf * 1024 + c * 512, 512)],
                            start=True, stop=True)
                    nc.scalar.activation(ptil[:, ds(hf * 1024, hw)], sc[:, :hw],
                                         AF.Exp, scale=ISCALE,
                                         accum_out=sums[:, ds(hf, 1)])
                s1 = stats.tile([P, 1], F32, tag="s1")
                nc.vector.tensor_reduce(s1, sums, axis=mybir.AxisListType.X, op=ALU.add)
                rcp = stats.tile([P, 1], F32, tag="rcp")
                nc.vector.reciprocal(rcp, s1)
                pout = pobuf.tile([P, s], F32, tag="pout")
                nc.vector.tensor_scalar_mul(pout, ptil, rcp)
                nc.gpsimd.dma_start(io["p_out"][h, ts(qt, P), :], pout)
                pb = pbbuf.tile([P, s], BF16, tag="pb")
                nc.vector.tensor_scalar_mul(pb, ptil, rcp)
                if qt % 4 == 0:
                    pts4 = ptbuf.tile([P, nt, 512], BF16, tag="pts4")
                nc.sync.dma_start_transpose(pts4[:, :, ds((qt % 4) * P, P)], pb)
                if qt % 4 == 3:
                    xps = psumX.tile([DV, 512], F32, tag="xps")
                    for kt in range(nt):
                        nc.tensor.matmul(xps, v4[:, kt, ds(h * DV, DV)],
                                         pts4[:, kt, :],
                                         start=(kt == 0), stop=(kt == nt - 1))
                    nc.vector.tensor_copy(xT4[ds(h * DV, DV), ds((qt // 4) * 512, 512)], xps)

    # ---- Stage C: partial output projection ----
    with tc.tile_pool(name="obuf", bufs=3) as obuf, \
         tc.tile_pool(name="psumO", bufs=2, space="PSUM") as psumO:
        for qt in range(nt):
            po = psumO.tile([P, D], F32, tag="po")
            nc.tensor.matmul(po, xT4[:, ts(qt, P)], wx_sb, start=True, stop=True)
            ot = obuf.tile([P, D], F32, tag="ot")
            nc.scalar.copy(ot, po)
            nc.sync.dma_start(io["x_out"][ts(qt, P), :], ot)


def build(s: int = S) -> bass.Bass:
    nc = bacc.Bacc("TRN2", target_bir_lowering=False, enable_partition_id=False)
    io = {}
    io["xq"] = nc.dram_tensor("xq", [s, D], F32, kind="ExternalInput").ap()
    io["xk"] = nc.dram_tensor("xk", [s, D], F32, kind="ExternalInput").ap()
    io["xv"] = nc.dram_tensor("xv", [s, D], F32, kind="ExternalInput").ap()
    io["wq"] = nc.dram_tensor("wq", [D, HPC * DK], BF16, kind="ExternalInput").ap()
    io["wk"] = nc.dram_tensor("wk", [D, HPC * DK], BF16, kind="ExternalInput").ap()
    io["wv"] = nc.dram_tensor("wv", [D, HPC * DV], BF16, kind="ExternalInput").ap()
    io["wx"] = nc.dram_tensor("wx", [HPC * DV, D], BF16, kind="ExternalInput").ap()
    io["bq"] = nc.dram_tensor("bq", [DK, HPC], F32, kind="ExternalInput").ap()
    io["bk"] = nc.dram_tensor("bk", [DK, HPC], F32, kind="ExternalInput").ap()
    io["bv"] = nc.dram_tensor("bv", [1, HPC * DV], BF16, kind="ExternalInput").ap()
    io["mb"] = nc.dram_tensor("mb", [2, s], BF16, kind="ExternalInput").ap()
    io["p_out"] = nc.dram_tensor("p_out", [HPC, s, s], F32, kind="ExternalOutput").ap()
    io["x_out"] = nc.dram_tensor("x_out", [s, D], F32, kind="ExternalOutput").ap()
    with tile.TileContext(nc) as tc:
        with ExitStack() as ctx:
            _emit(ctx, tc, io, s)
    nc.compile()
    return nc


def make_in_maps(query, key, value, mask, Wq, bq, Wk, bk, Wv, bv, Wx, bx):
    """Build the 8 per-core input dicts. Core 2*b+g -> (batch b, head group g)."""
    f = np.float32
    in_maps = []
    for core in range(NCORES):
        b, g = divmod(core, 2)
        qs = slice(g * HPC * DK, (g + 1) * HPC * DK)
        vs = slice(g * HPC * DV, (g + 1) * HPC * DV)
        mbrow = (np.asarray(mask[b, 0], f) - 1.0) * -NEG  # 0 -> -1e9, 1 -> 0
        mb = np.stack([mbrow, np.ones(S, f)])
        in_maps.append({
            "xq": np.ascontiguousarray(query[b], f),
            "xk": np.ascontiguousarray(key[b], f),
            "xv": np.ascontiguousarray(value[b], f),
            "wq": np.ascontiguousarray(Wq[:, qs]).astype(NPBF),
            "wk": np.ascontiguousarray(Wk[:, qs]).astype(NPBF),
            "wv": np.ascontiguousarray(Wv[:, vs]).astype(NPBF),
            "wx": np.ascontiguousarray(Wx[vs, :]).astype(NPBF),
            "bq": np.ascontiguousarray(np.asarray(bq[qs], f).reshape(HPC, DK).T),
            "bk": np.ascontiguousarray(np.asarray(bk[qs], f).reshape(HPC, DK).T),
            "bv": np.asarray(bv[vs], f).reshape(1, HPC * DV).astype(NPBF),
            "mb": np.ascontiguousarray(mb).astype(NPBF),
        })
    return in_maps


_NC_CACHE = {}


def _get_nc():
    if "nc" not in _NC_CACHE:
        _NC_CACHE["nc"] = build(S)
    return _NC_CACHE["nc"]


def run(in_maps, trace=False, **kw):
    return run_bass_kernel_spmd(_get_nc(), in_maps, core_ids=list(range(NCORES)),
                                trace=trace, **kw)


def kernel(query, key, value, mask, Wq, bq, Wk, bk, Wv, bv, Wx, bx):
    in_maps = make_in_maps(query, key, value, mask, Wq, bq, Wk, bk, Wv, bv, Wx, bx)
    res = run(in_maps).results
    x = np.zeros((B, S, D), np.float32)
    p_attn = np.zeros((B, H, S, S), np.float32)
    for core in range(NCORES):
        b, g = divmod(core, 2)
        p_attn[b, g * HPC : (g + 1) * HPC] = res[core]["p_out"]
        x[b] += res[core]["x_out"]
    x += np.asarray(bx, np.float32)
    return x, p_attn


# revision 9
# speedup vs baseline: 1.0904x; 1.0904x over previous
"""Trainium2 Bass kernel for nn_MultiHeadAttention (B=4, S=2048, D=512, H=8, DK=12, DV=32).

Sharding: 8 cores = (batch b in 0..3) x (head-group g in 0..1, 4 heads each).
Each core computes, for its batch and its 4 heads:
  q/k/v projections, scores, masked softmax, p_attn (returned), x = p @ v,
  and a PARTIAL output projection x @ Wx[group rows].
Host sums the two partial outputs per batch (the "all-reduce") and adds bx.

Per-core kernel design notes:
  * All matmul operands are bf16 (fp32 matmuls are split into LO/HI passes on
    TRN2 at 2x the cost). Scores/exp/softmax normalization stay fp32.
  * Scores use a K=13 contraction: rows 0-11 are head dims, row 12 carries
    ones (qT side) x mask-bias (-1e9 or 0, kT side) so masking is folded into
    the score matmul for free.
  * exp runs on the scalar engine over a whole [128, 2048] stripe with
    scale=1/sqrt(DK) and accum_out producing row sums; softmax skips the
    max-subtraction (scores are O(1); exp((s-1e9)*scale) underflows to 0.0
    exactly like the reference).
  * p is normalized on DVE, DMA'd out in fp32, converted to bf16, and
    transposed via the DMA xbar (2-byte dtype) straight into SBUF as the
    moving operand of the x = p @ v matmul. No PE transposes, no PSUM copies.
"""

import math
from contextlib import ExitStack

import ml_dtypes
import numpy as np

import concourse.bass as bass
import concourse.tile as tile
from concourse import bacc, mybir
from concourse.bass import ts, ds
from concourse.bass_utils import run_bass_kernel_spmd
from concourse.masks import make_identity

AF = mybir.ActivationFunctionType
ALU = mybir.AluOpType
F32 = mybir.dt.float32
BF16 = mybir.dt.bfloat16
NPBF = ml_dtypes.bfloat16

B, S, D = 4, 2048, 512
H, DK, DV = 8, 12, 32
HPC = 4  # heads per core
NCORES = 8
P = 128
ISCALE = 1.0 / math.sqrt(DK)
NEG = -1.0e9


def _emit(ctx: ExitStack, tc: "tile.TileContext", io: dict, s: int) -> None:
    nc = tc.nc
    nt = s // P          # q/k row tiles
    nch = s // 512       # 512-wide column chunks of the score row
    dch = D // P         # contraction chunks over D

    consts = ctx.enter_context(tc.tile_pool(name="consts", bufs=1))
    persist = ctx.enter_context(tc.tile_pool(name="persist", bufs=1))

    wq_sb = consts.tile([P, dch, HPC * DK], BF16)
    nc.sync.dma_start(wq_sb, io["wq"].rearrange("(c p) m -> p c m", p=P))
    wk_sb = consts.tile([P, dch, HPC * DK], BF16)
    nc.sync.dma_start(wk_sb, io["wk"].rearrange("(c p) m -> p c m", p=P))
    wv_sb = consts.tile([P, dch, HPC * DV], BF16)
    nc.sync.dma_start(wv_sb, io["wv"].rearrange("(c p) m -> p c m", p=P))
    wx_sb = consts.tile([P, D], BF16)
    nc.sync.dma_start(wx_sb, io["wx"])
    bq_sb = consts.tile([DK, HPC], F32)
    nc.sync.dma_start(bq_sb, io["bq"])
    bk_sb = consts.tile([DK, HPC], F32)
    nc.sync.dma_start(bk_sb, io["bk"])
    bv_sb = consts.tile([1, HPC * DV], BF16)
    nc.sync.dma_start(bv_sb, io["bv"])
    ones_sb = consts.tile([1, P], BF16)
    nc.vector.memset(ones_sb, 1.0)
    ident = consts.tile([P, P], F32)
    make_identity(nc, ident)

    # qTe/kTe: rows 0..11 = projected head dims (transposed), row 12 = ones /
    # mask-bias so the score matmul applies the mask.
    qTe = persist.tile([DK + 1, HPC, s], BF16)
    kTe = persist.tile([DK + 1, HPC, s], BF16)
    v4 = persist.tile([P, nt, HPC * DV], BF16)   # [k_local, k_tile, head*dv]
    xT4 = persist.tile([P, s], BF16)             # [head*dv, q]

    for h in range(HPC):
        nc.sync.dma_start(qTe[DK : DK + 1, h, :], io["mb"][1:2, :])
        nc.sync.dma_start(kTe[DK : DK + 1, h, :], io["mb"][0:1, :])

    # ---- Stage A: transpose inputs, project q/k/v ----
    with tc.tile_pool(name="loads", bufs=3) as loads, \
         tc.tile_pool(name="rawT", bufs=2) as rawT, \
         tc.tile_pool(name="psumA", bufs=2, space="PSUM") as psumA, \
         tc.tile_pool(name="psumP", bufs=2, space="PSUM") as psumP:

        def transpose_in(src):
            """Load [s, D] DRAM tensor, return SBUF [P, dch, s] bf16 transpose."""
            dstT = rawT.tile([P, dch, s], BF16, tag="rawT")
            for t in range(nt):
                xt = loads.tile([P, D], F32, tag="xt")
                nc.gpsimd.dma_start(xt, src[ts(t, P), :])
                tp = psumA.tile([P, dch, P], F32, tag="tp")
                for c in range(dch):
                    nc.tensor.transpose(tp[:, c, :], xt[:, ts(c, P)], ident)
                nc.vector.tensor_copy(dstT[:, :, ts(t, P)], tp)
            return dstT

        qraw = transpose_in(io["xq"])
        for h in range(HPC):
            for n in range(s // 512):
                pq = psumP.tile([DK, 512], F32, tag="pqk")
                for c in range(dch):
                    nc.tensor.matmul(
                        pq, wq_sb[:, c, ds(h * DK, DK)], qraw[:, c, ds(n * 512, 512)],
                        start=(c == 0), stop=(c == dch - 1))
                nc.scalar.activation(
                    qTe[:DK, h, ds(n * 512, 512)], pq, AF.Identity,
                    bias=bq_sb[:, h : h + 1], scale=1.0)

        kraw = transpose_in(io["xk"])
        for h in range(HPC):
            for n in range(s // 512):
                pk = psumP.tile([DK, 512], F32, tag="pqk")
                for c in range(dch):
                    nc.tensor.matmul(
                        pk, wk_sb[:, c, ds(h * DK, DK)], kraw[:, c, ds(n * 512, 512)],
                        start=(c == 0), stop=(c == dch - 1))
                nc.scalar.activation(
                    kTe[:DK, h, ds(n * 512, 512)], pk, AF.Identity,
                    bias=bk_sb[:, h : h + 1], scale=1.0)

        vraw = transpose_in(io["xv"])
        for t in range(nt):
            pv = psumP.tile([P, HPC * DV], F32, tag="pv")
            for c in range(dch):
                nc.tensor.matmul(pv, vraw[:, c, ts(t, P)], wv_sb[:, c, :],
                                 start=(c == 0), stop=False)
            nc.tensor.matmul(pv, ones_sb, bv_sb, start=False, stop=True)
            nc.vector.tensor_copy(v4[:, t, :], pv)

    # ---- Stage B: attention per (head, q-stripe) ----
    with tc.tile_pool(name="pbuf", bufs=3) as pbuf, \
         tc.tile_pool(name="pobuf", bufs=4) as pobuf, \
         tc.tile_pool(name="pbbuf", bufs=3) as pbbuf, \
         tc.tile_pool(name="ptbuf", bufs=2) as ptbuf, \
         tc.tile_pool(name="stats", bufs=8) as stats, \
         tc.tile_pool(name="psumS", bufs=3, space="PSUM") as psumS, \
         tc.tile_pool(name="psumX", bufs=2, space="PSUM") as psumX:
        for h in range(HPC):
            pts4 = None
            for qt in range(nt):
                nhf = max(1, s // 1024)
                sums = stats.tile([P, nhf], F32, tag="sums")
                ptil = pbuf.tile([P, s], BF16, tag="ptil")
                for hf in range(nhf):
                    hw = min(1024, s)
                    sc = psumS.tile([P, 1024], F32, tag="sc")
                    for c in range(hw // 512):
                        nc.tensor.matmul(
                            sc[:, ds(c * 512, 512)],
                            qTe[:, h, ts(qt, P)],
                            kTe[:, h, ds(hf * 1024 + c * 512, 512)],
                            start=True, stop=True)
                    nc.scalar.activation(ptil[:, ds(hf * 1024, hw)], sc[:, :hw],
                                         AF.Exp, scale=ISCALE,
                                         accum_out=sums[:, ds(hf, 1)])
                s1 = stats.tile([P, 1], F32, tag="s1")
                nc.vector.tensor_reduce(s1, sums, axis=mybir.AxisListType.X, op=ALU.add)
                rcp = stats.tile([P, 1], F32, tag="rcp")
                nc.vector.reciprocal(rcp, s1)
                pb = pbbuf.tile([P, s], BF16, tag="pb")
                nc.vector.tensor_scalar_mul(pb, ptil, rcp)
                nc.gpsimd.dma_start(io["p_out"][h, ts(qt, P), :], pb)
                if qt % 4 == 0:
                    pts4 = ptbuf.tile([P, nt, 512], BF16, tag="pts4")
                nc.sync.dma_start_transpose(pts4[:, :, ds((qt % 4) * P, P)], pb)
                if qt % 4 == 3:
                    xps = psumX.tile([DV, 512], F32, tag="xps")
                    for kt in range(nt):
                        nc.tensor.matmul(xps, v4[:, kt, ds(h * DV, DV)],
                                         pts4[:, kt, :],
                                         start=(kt == 0), stop=(kt == nt - 1))
                    nc.vector.tensor_copy(xT4[ds(h * DV, DV), ds((qt // 4) * 512, 512)], xps)

    # ---- Stage C: partial output projection ----
    with tc.tile_pool(name="obuf", bufs=3) as obuf, \
         tc.tile_pool(name="psumO", bufs=2, space="PSUM") as psumO:
        for qt in range(nt):
            po = psumO.tile([P, D], F32, tag="po")
            nc.tensor.matmul(po, xT4[:, ts(qt, P)], wx_sb, start=True, stop=True)
            ot = obuf.tile([P, D], F32, tag="ot")
            nc.scalar.copy(ot, po)
            nc.sync.dma_start(io["x_out"][ts(qt, P), :], ot)


def build(s: int = S) -> bass.Bass:
    nc = bacc.Bacc("TRN2", target_bir_lowering=False, enable_partition_id=False)
    io = {}
    io["xq"] = nc.dram_tensor("xq", [s, D], F32, kind="ExternalInput").ap()
    io["xk"] = nc.dram_tensor("xk", [s, D], F32, kind="ExternalInput").ap()
    io["xv"] = nc.dram_tensor("xv", [s, D], F32, kind="ExternalInput").ap()
    io["wq"] = nc.dram_tensor("wq", [D, HPC * DK], BF16, kind="ExternalInput").ap()
    io["wk"] = nc.dram_tensor("wk", [D, HPC * DK], BF16, kind="ExternalInput").ap()
    io["wv"] = nc.dram_tensor("wv", [D, HPC * DV], BF16, kind="ExternalInput").ap()
    io["wx"] = nc.dram_tensor("wx", [HPC * DV, D], BF16, kind="ExternalInput").ap()
    io["bq"] = nc.dram_tensor("bq", [DK, HPC], F32, kind="ExternalInput").ap()
    io["bk"] = nc.dram_tensor("bk", [DK, HPC], F32, kind="ExternalInput").ap()
    io["bv"] = nc.dram_tensor("bv", [1, HPC * DV], BF16, kind="ExternalInput").ap()
    io["mb"] = nc.dram_tensor("mb", [2, s], BF16, kind="ExternalInput").ap()
    io["p_out"] = nc.dram_tensor("p_out", [HPC, s, s], BF16, kind="ExternalOutput").ap()
    io["x_out"] = nc.dram_tensor("x_out", [s, D], F32, kind="ExternalOutput").ap()
    with tile.TileContext(nc) as tc:
        with ExitStack() as ctx:
            _emit(ctx, tc, io, s)
    nc.compile()
    return nc


def make_in_maps(query, key, value, mask, Wq, bq, Wk, bk, Wv, bv, Wx, bx):
    """Build the 8 per-core input dicts. Core 2*b+g -> (batch b, head group g)."""
    f = np.float32
    in_maps = []
    for core in range(NCORES):
        b, g = divmod(core, 2)
        qs = slice(g * HPC * DK, (g + 1) * HPC * DK)
        vs = slice(g * HPC * DV, (g + 1) * HPC * DV)
        mbrow = (np.asarray(mask[b, 0], f) - 1.0) * -NEG  # 0 -> -1e9, 1 -> 0
        mb = np.stack([mbrow, np.ones(S, f)])
        in_maps.append({
            "xq": np.ascontiguousarray(query[b], f),
            "xk": np.ascontiguousarray(key[b], f),
            "xv": np.ascontiguousarray(value[b], f),
            "wq": np.ascontiguousarray(Wq[:, qs]).astype(NPBF),
            "wk": np.ascontiguousarray(Wk[:, qs]).astype(NPBF),
            "wv": np.ascontiguousarray(Wv[:, vs]).astype(NPBF),
            "wx": np.ascontiguousarray(Wx[vs, :]).astype(NPBF),
            "bq": np.ascontiguousarray(np.asarray(bq[qs], f).reshape(HPC, DK).T),
            "bk": np.ascontiguousarray(np.asarray(bk[qs], f).reshape(HPC, DK).T),
            "bv": np.asarray(bv[vs], f).reshape(1, HPC * DV).astype(NPBF),
            "mb": np.ascontiguousarray(mb).astype(NPBF),
        })
    return in_maps


_NC_CACHE = {}


def _get_nc():
    if "nc" not in _NC_CACHE:
        _NC_CACHE["nc"] = build(S)
    return _NC_CACHE["nc"]


def run(in_maps, trace=False, **kw):
    return run_bass_kernel_spmd(_get_nc(), in_maps, core_ids=list(range(NCORES)),
                                trace=trace, **kw)


def kernel(query, key, value, mask, Wq, bq, Wk, bk, Wv, bv, Wx, bx):
    in_maps = make_in_maps(query, key, value, mask, Wq, bq, Wk, bk, Wv, bv, Wx, bx)
    res = run(in_maps).results
    x = np.zeros((B, S, D), np.float32)
    p_attn = np.zeros((B, H, S, S), np.float32)
    for core in range(NCORES):
        b, g = divmod(core, 2)
        p_attn[b, g * HPC : (g + 1) * HPC] = np.asarray(res[core]["p_out"], np.float32)
        x[b] += res[core]["x_out"]
    x += np.asarray(bx, np.float32)
    return x, p_attn


# revision 11
# speedup vs baseline: 1.6703x; 1.5318x over previous
"""Trainium2 Bass kernel for nn_MultiHeadAttention (B=4, S=2048, D=512, H=8, DK=12, DV=32).

Sharding: 8 cores = (batch b in 0..3) x (head-group g in 0..1, 4 heads each).
Each core computes, for its batch and its 4 heads:
  q/k/v projections, scores, masked softmax, p_attn (returned), x = p @ v,
  and a PARTIAL output projection x @ Wx[group rows].
Host sums the two partial outputs per batch (the "all-reduce") and adds bx.

Per-core kernel design notes:
  * All matmul operands are bf16 (fp32 matmuls are split into LO/HI passes on
    TRN2 at 2x the cost). Scores/exp/softmax normalization stay fp32.
  * Scores use a K=13 contraction: rows 0-11 are head dims, row 12 carries
    ones (qT side) x mask-bias (-1e9 or 0, kT side) so masking is folded into
    the score matmul for free.
  * exp runs on the scalar engine over a whole [128, 2048] stripe with
    scale=1/sqrt(DK) and accum_out producing row sums; softmax skips the
    max-subtraction (scores are O(1); exp((s-1e9)*scale) underflows to 0.0
    exactly like the reference).
  * p is normalized on DVE, DMA'd out in fp32, converted to bf16, and
    transposed via the DMA xbar (2-byte dtype) straight into SBUF as the
    moving operand of the x = p @ v matmul. No PE transposes, no PSUM copies.
"""

import math
from contextlib import ExitStack

import ml_dtypes
import numpy as np

import concourse.bass as bass
import concourse.tile as tile
from concourse import bacc, mybir
from concourse.bass import ts, ds
from concourse.bass_utils import run_bass_kernel_spmd
from concourse.masks import make_identity

AF = mybir.ActivationFunctionType
ALU = mybir.AluOpType
F32 = mybir.dt.float32
BF16 = mybir.dt.bfloat16
NPBF = ml_dtypes.bfloat16

B, S, D = 4, 2048, 512
H, DK, DV = 8, 12, 32
HPC = 4  # heads per core
NCORES = 8
P = 128
ISCALE = 1.0 / math.sqrt(DK)
NEG = -1.0e9


def _emit(ctx: ExitStack, tc: "tile.TileContext", io: dict, s: int) -> None:
    nc = tc.nc
    nt = s // P          # q/k row tiles
    nch = s // 512       # 512-wide column chunks of the score row
    dch = D // P         # contraction chunks over D

    consts = ctx.enter_context(tc.tile_pool(name="consts", bufs=1))
    persist = ctx.enter_context(tc.tile_pool(name="persist", bufs=1))

    wq_sb = consts.tile([P, dch, HPC * DK], BF16)
    nc.sync.dma_start(wq_sb, io["wq"].rearrange("(c p) m -> p c m", p=P))
    wk_sb = consts.tile([P, dch, HPC * DK], BF16)
    nc.sync.dma_start(wk_sb, io["wk"].rearrange("(c p) m -> p c m", p=P))
    wv_sb = consts.tile([P, dch, HPC * DV], BF16)
    nc.sync.dma_start(wv_sb, io["wv"].rearrange("(c p) m -> p c m", p=P))
    wx_sb = consts.tile([P, D], BF16)
    nc.sync.dma_start(wx_sb, io["wx"])
    bq_sb = consts.tile([DK, HPC], F32)
    nc.sync.dma_start(bq_sb, io["bq"])
    bk_sb = consts.tile([DK, HPC], F32)
    nc.sync.dma_start(bk_sb, io["bk"])
    bv_sb = consts.tile([1, HPC * DV], BF16)
    nc.sync.dma_start(bv_sb, io["bv"])
    ones_sb = consts.tile([1, P], BF16)
    nc.vector.memset(ones_sb, 1.0)
    ident = consts.tile([P, P], F32)
    make_identity(nc, ident)
    ident_bf = consts.tile([P, P], BF16)
    make_identity(nc, ident_bf)

    # qTe/kTe: rows 0..11 = projected head dims (transposed), row 12 = ones /
    # mask-bias so the score matmul applies the mask.
    qTe = persist.tile([DK + 1, HPC, s], BF16)
    kTe = persist.tile([DK + 1, HPC, s], BF16)
    v4 = persist.tile([P, nt, HPC * DV], BF16)   # [k_local, k_tile, head*dv]
    xT4 = persist.tile([P, s], BF16)             # [head*dv, q]

    for h in range(HPC):
        nc.sync.dma_start(qTe[DK : DK + 1, h, :], io["mb"][1:2, :])
        nc.sync.dma_start(kTe[DK : DK + 1, h, :], io["mb"][0:1, :])

    # ---- Stage A: transpose inputs, project q/k/v ----
    with tc.tile_pool(name="loads", bufs=3) as loads, \
         tc.tile_pool(name="rawT", bufs=2) as rawT, \
         tc.tile_pool(name="psumA", bufs=2, space="PSUM") as psumA, \
         tc.tile_pool(name="psumP", bufs=2, space="PSUM") as psumP:

        def transpose_in(src):
            """Load [s, D] DRAM tensor, return SBUF [P, dch, s] bf16 transpose."""
            dstT = rawT.tile([P, dch, s], BF16, tag="rawT")
            for t in range(nt):
                xt = loads.tile([P, D], F32, tag="xt")
                nc.gpsimd.dma_start(xt, src[ts(t, P), :])
                tp = psumA.tile([P, dch, P], F32, tag="tp")
                for c in range(dch):
                    nc.tensor.transpose(tp[:, c, :], xt[:, ts(c, P)], ident)
                nc.vector.tensor_copy(dstT[:, :, ts(t, P)], tp)
            return dstT

        qraw = transpose_in(io["xq"])
        for h in range(HPC):
            for n in range(s // 512):
                pq = psumP.tile([DK, 512], F32, tag="pqk")
                for c in range(dch):
                    nc.tensor.matmul(
                        pq, wq_sb[:, c, ds(h * DK, DK)], qraw[:, c, ds(n * 512, 512)],
                        start=(c == 0), stop=(c == dch - 1))
                nc.scalar.activation(
                    qTe[:DK, h, ds(n * 512, 512)], pq, AF.Identity,
                    bias=bq_sb[:, h : h + 1], scale=1.0)

        kraw = transpose_in(io["xk"])
        for h in range(HPC):
            for n in range(s // 512):
                pk = psumP.tile([DK, 512], F32, tag="pqk")
                for c in range(dch):
                    nc.tensor.matmul(
                        pk, wk_sb[:, c, ds(h * DK, DK)], kraw[:, c, ds(n * 512, 512)],
                        start=(c == 0), stop=(c == dch - 1))
                nc.scalar.activation(
                    kTe[:DK, h, ds(n * 512, 512)], pk, AF.Identity,
                    bias=bk_sb[:, h : h + 1], scale=1.0)

        vraw = transpose_in(io["xv"])
        for t in range(nt):
            pv = psumP.tile([P, HPC * DV], F32, tag="pv")
            for c in range(dch):
                nc.tensor.matmul(pv, vraw[:, c, ts(t, P)], wv_sb[:, c, :],
                                 start=(c == 0), stop=False)
            nc.tensor.matmul(pv, ones_sb, bv_sb, start=False, stop=True)
            nc.vector.tensor_copy(v4[:, t, :], pv)

    # ---- Stage B: attention per (head, q-stripe) ----
    with tc.tile_pool(name="pbuf", bufs=3) as pbuf, \
         tc.tile_pool(name="pobuf", bufs=4) as pobuf, \
         tc.tile_pool(name="pbbuf", bufs=3) as pbbuf, \
         tc.tile_pool(name="ptbuf", bufs=2) as ptbuf, \
         tc.tile_pool(name="stats", bufs=8) as stats, \
         tc.tile_pool(name="psumS", bufs=2, space="PSUM") as psumS, \
         tc.tile_pool(name="psumT", bufs=2, space="PSUM") as psumT, \
         tc.tile_pool(name="psumX", bufs=2, space="PSUM") as psumX:
        for h in range(HPC):
            pts4 = None
            for qt in range(nt):
                nhf = max(1, s // 1024)
                sums = stats.tile([P, nhf], F32, tag="sums")
                ptil = pbuf.tile([P, s], BF16, tag="ptil")
                for hf in range(nhf):
                    hw = min(1024, s)
                    sc = psumS.tile([P, 1024], F32, tag="sc")
                    for c in range(hw // 512):
                        nc.tensor.matmul(
                            sc[:, ds(c * 512, 512)],
                            qTe[:, h, ts(qt, P)],
                            kTe[:, h, ds(hf * 1024 + c * 512, 512)],
                            start=True, stop=True)
                    nc.scalar.activation(ptil[:, ds(hf * 1024, hw)], sc[:, :hw],
                                         AF.Exp, scale=ISCALE,
                                         accum_out=sums[:, ds(hf, 1)])
                s1 = stats.tile([P, 1], F32, tag="s1")
                nc.vector.tensor_reduce(s1, sums, axis=mybir.AxisListType.X, op=ALU.add)
                rcp = stats.tile([P, 1], F32, tag="rcp")
                nc.vector.reciprocal(rcp, s1)
                pb = pbbuf.tile([P, s], BF16, tag="pb")
                nc.vector.tensor_scalar_mul(pb, ptil, rcp)
                nc.gpsimd.dma_start(io["p_out"][h, ts(qt, P), :], pb)
                if qt % 4 == 0:
                    pts4 = ptbuf.tile([P, nt, 512], BF16, tag="pts4")
                for k0 in range(0, nt, 8):
                    kw = min(8, nt - k0)
                    ptp = psumT.tile([P, 8, P], BF16, tag="ptp")
                    for j in range(kw):
                        nc.tensor.transpose(ptp[:, j, :], pb[:, ds((k0 + j) * P, P)],
                                            ident_bf)
                    nc.vector.tensor_copy(
                        pts4[:, ds(k0, kw), ds((qt % 4) * P, P)], ptp[:, :kw, :])
                if qt % 4 == 3:
                    xps = psumX.tile([DV, 512], F32, tag="xps")
                    for kt in range(nt):
                        nc.tensor.matmul(xps, v4[:, kt, ds(h * DV, DV)],
                                         pts4[:, kt, :],
                                         start=(kt == 0), stop=(kt == nt - 1))
                    nc.vector.tensor_copy(xT4[ds(h * DV, DV), ds((qt // 4) * 512, 512)], xps)

    # ---- Stage C: partial output projection ----
    with tc.tile_pool(name="obuf", bufs=3) as obuf, \
         tc.tile_pool(name="psumO", bufs=2, space="PSUM") as psumO:
        for qt in range(nt):
            po = psumO.tile([P, D], F32, tag="po")
            nc.tensor.matmul(po, xT4[:, ts(qt, P)], wx_sb, start=True, stop=True)
            ot = obuf.tile([P, D], F32, tag="ot")
            nc.scalar.copy(ot, po)
            nc.sync.dma_start(io["x_out"][ts(qt, P), :], ot)


def build(s: int = S) -> bass.Bass:
    nc = bacc.Bacc("TRN2", target_bir_lowering=False, enable_partition_id=False)
    io = {}
    io["xq"] = nc.dram_tensor("xq", [s, D], F32, kind="ExternalInput").ap()
    io["xk"] = nc.dram_tensor("xk", [s, D], F32, kind="ExternalInput").ap()
    io["xv"] = nc.dram_tensor("xv", [s, D], F32, kind="ExternalInput").ap()
    io["wq"] = nc.dram_tensor("wq", [D, HPC * DK], BF16, kind="ExternalInput").ap()
    io["wk"] = nc.dram_tensor("wk", [D, HPC * DK], BF16, kind="ExternalInput").ap()
    io["wv"] = nc.dram_tensor("wv", [D, HPC * DV], BF16, kind="ExternalInput").ap()
    io["wx"] = nc.dram_tensor("wx", [HPC * DV, D], BF16, kind="ExternalInput").ap()
    io["bq"] = nc.dram_tensor("bq", [DK, HPC], F32, kind="ExternalInput").ap()
    io["bk"] = nc.dram_tensor("bk", [DK, HPC], F32, kind="ExternalInput").ap()
    io["bv"] = nc.dram_tensor("bv", [1, HPC * DV], BF16, kind="ExternalInput").ap()
    io["mb"] = nc.dram_tensor("mb", [2, s], BF16, kind="ExternalInput").ap()
    io["p_out"] = nc.dram_tensor("p_out", [HPC, s, s], BF16, kind="ExternalOutput").ap()
    io["x_out"] = nc.dram_tensor("x_out", [s, D], F32, kind="ExternalOutput").ap()
    with tile.TileContext(nc) as tc:
        with ExitStack() as ctx:
            _emit(ctx, tc, io, s)
    nc.compile()
    return nc


def make_in_maps(query, key, value, mask, Wq, bq, Wk, bk, Wv, bv, Wx, bx):
    """Build the 8 per-core input dicts. Core 2*b+g -> (batch b, head group g)."""
    f = np.float32
    in_maps = []
    for core in range(NCORES):
        b, g = divmod(core, 2)
        qs = slice(g * HPC * DK, (g + 1) * HPC * DK)
        vs = slice(g * HPC * DV, (g + 1) * HPC * DV)
        mbrow = (np.asarray(mask[b, 0], f) - 1.0) * -NEG  # 0 -> -1e9, 1 -> 0
        mb = np.stack([mbrow, np.ones(S, f)])
        in_maps.append({
            "xq": np.ascontiguousarray(query[b], f),
            "xk": np.ascontiguousarray(key[b], f),
            "xv": np.ascontiguousarray(value[b], f),
            "wq": np.ascontiguousarray(Wq[:, qs]).astype(NPBF),
            "wk": np.ascontiguousarray(Wk[:, qs]).astype(NPBF),
            "wv": np.ascontiguousarray(Wv[:, vs]).astype(NPBF),
            "wx": np.ascontiguousarray(Wx[vs, :]).astype(NPBF),
            "bq": np.ascontiguousarray(np.asarray(bq[qs], f).reshape(HPC, DK).T),
            "bk": np.ascontiguousarray(np.asarray(bk[qs], f).reshape(HPC, DK).T),
            "bv": np.asarray(bv[vs], f).reshape(1, HPC * DV).astype(NPBF),
            "mb": np.ascontiguousarray(mb).astype(NPBF),
        })
    return in_maps


_NC_CACHE = {}


def _get_nc():
    if "nc" not in _NC_CACHE:
        _NC_CACHE["nc"] = build(S)
    return _NC_CACHE["nc"]


def run(in_maps, trace=False, **kw):
    return run_bass_kernel_spmd(_get_nc(), in_maps, core_ids=list(range(NCORES)),
                                trace=trace, **kw)


def kernel(query, key, value, mask, Wq, bq, Wk, bk, Wv, bv, Wx, bx):
    in_maps = make_in_maps(query, key, value, mask, Wq, bq, Wk, bk, Wv, bv, Wx, bx)
    res = run(in_maps).results
    x = np.zeros((B, S, D), np.float32)
    p_attn = np.zeros((B, H, S, S), np.float32)
    for core in range(NCORES):
        b, g = divmod(core, 2)
        p_attn[b, g * HPC : (g + 1) * HPC] = np.asarray(res[core]["p_out"], np.float32)
        x[b] += res[core]["x_out"]
    x += np.asarray(bx, np.float32)
    return x, p_attn
